# revision 12
# baseline (speedup 1.0000x reference)
"""Distributed Bass kernel for nn_Attention (dense transformer block with the
q=k=v source bug) on 8 TRN2 NeuronCores.

Sharding: tensor-parallel over heads (Megatron-style). Core i owns heads
(2i, 2i+1). Per core:
  1. kproj: KT[d, t] = (x @ W_k_slice).T from host-transposed 1MB x^T chunks;
     KN[k-token, d] via DMA-xbar transposes (off the PE).
  2. Attention per (batch, 1024-query-span, head, 128-key-block): S-pair
     matmuls (2x N=512, f32 PSUM bank limit), one N=1024 Exp activation
     whose accum_out gives softmax denominators for free (E is symmetric
     since q=k, so free-axis row sums == column sums), then O^T
     accumulation col-packed 2 heads per PSUM tile. The kb loop is
     software-pipelined in EMISSION order (S(kb+1) before O(kb)) because
     engine streams execute strictly in order - otherwise the PE stalls at
     O(kb)'s wait-for-exp while S(kb+1) is ready behind it. ACT is the
     roofline: 128 exps x 1.11us + 128 accum drains x 0.28us ~= 178us.
  3. kproj of batch 1 is drip-fed one matmul per kb slot inside batch 0's
     attention segments so it never bubbles the exp stream.
  4. Normalize per batch: fast reciprocal, PE transpose, DRAM
     partition-broadcast, in-place multiply.
  5. AllToAll per (batch, 128-token half); out-projections are emitted
     after the comms so they execute during the A2A waits (HAM warm).

PSUM: S 2x2 banks, ot 2, kproj/proj scratch 2x1.
"""

import numpy as np

import concourse.bass as bass
import concourse.tile as tile
from concourse import bacc, mybir
from concourse.bass_utils import run_bass_kernel_spmd
from concourse.masks import make_identity

N_CORES = 8
B, L, D = 2, 2048, 1024
H, HD = 16, 64
HPC = H // N_CORES  # heads per core = 2
DC = HPC * HD  # head-dim columns per core = 128
TPB = L // N_CORES  # tokens per batch per core (proj stage) = 256
F32 = mybir.dt.float32
BF16 = mybir.dt.bfloat16
QS = 1024  # query span per (head, kb) exp tile
KB = L // 128  # 16 key blocks per batch
NH = 2  # token halves per batch for the A2A/proj pipeline
TH = TPB // NH  # 128 tokens per half


def build():
    nc = bacc.Bacc("TRN2", target_bir_lowering=False, debug=False, num_devices=N_CORES)
    xt = nc.dram_tensor("xt", [B, D, L], BF16, kind="ExternalInput")
    wk = nc.dram_tensor("wk", [D, DC], BF16, kind="ExternalInput")
    bk = nc.dram_tensor("bk", [DC, 1], F32, kind="ExternalInput")
    wp = nc.dram_tensor("wp", [D, D], BF16, kind="ExternalInput")
    bp = nc.dram_tensor("bp", [D], F32, kind="ExternalInput")
    out = nc.dram_tensor("out", [B, TPB, D], F32, kind="ExternalOutput")

    xt_v = xt.ap().rearrange("b (dc p) t -> b p dc t", p=128)  # [B, 128, 8, L]
    wk_v = wk.ap().rearrange("(dc p) m -> p dc m", p=128)  # [128, 8, DC]
    wp_v = wp.ap().rearrange("(dc p) n -> p dc n", p=128)  # [128, 8, D]

    with tile.TileContext(nc) as tc:
        with (
            tc.tile_pool(name="consts", bufs=1) as consts,
            tc.tile_pool(name="big", bufs=1) as big,
            tc.tile_pool(name="xtp", bufs=4) as xtp,
            tc.tile_pool(name="fpool", bufs=3) as fpool,
            tc.tile_pool(name="small", bufs=4) as small,
            tc.tile_pool(name="rrp", bufs=2) as rrp,
            tc.tile_pool(name="ptp", bufs=2) as ptp,
            tc.tile_pool(name="yp", bufs=2) as yp,
            tc.tile_pool(name="spool", bufs=2, space="PSUM") as spool,
            tc.tile_pool(name="otp", bufs=1, space="PSUM") as otp,
            tc.tile_pool(name="pkp", bufs=2, space="PSUM") as pkp,
            tc.tile_pool(name="dram", bufs=1, space="DRAM") as dram,
        ):
            # ---- constants ----
            wk_sb = consts.tile([128, 8, DC], BF16)
            nc.sync.dma_start(wk_sb[:], wk_v)
            bk_sb = consts.tile([128, 1], F32)
            nc.sync.dma_start(bk_sb[:], bk.ap())
            wp_sb = consts.tile([128, 8, D], BF16)
            nc.sync.dma_start(wp_sb[:], wp_v)
            bp_rep = consts.tile([128, D], F32)
            nc.sync.dma_start(bp_rep[:], bp.ap().partition_broadcast(128))
            identb = consts.tile([128, 128], BF16)
            make_identity(nc, identb[:])

            # persistent activations
            KT = [big.tile([128, L], BF16, name=f"kt{b}") for b in range(B)]
            KN = [big.tile([128, KB, 128], BF16, name=f"kn{b}") for b in range(B)]
            OTn = [big.tile([128, L], BF16, name=f"otn{b}") for b in range(B)]
            saccs = [big.tile([128, HPC, KB, 2], F32, name=f"sacc{b}") for b in range(B)]

            rsd = [dram.tile([HPC * KB, 128], BF16, name=f"rsd{b}") for b in range(B)]
            cc_in = [[dram.tile([N_CORES * 128, TH], BF16, name=f"cc_in{b}_{h}")
                      for h in range(NH)] for b in range(B)]
            cc_out = [[dram.tile([N_CORES * 128, TH], BF16, name=f"cc_out{b}_{h}")
                       for h in range(NH)] for b in range(B)]

            def load_xt(b):
                tiles = []
                for c in range(4):
                    xt_t = xtp.tile([128, 8, 512], BF16, tag="xt")
                    nc.gpsimd.dma_start(xt_t[:], xt_v[b, :, :, c * 512:(c + 1) * 512])
                    tiles.append(xt_t)
                return tiles

            def kproj_chunk_tail(b, c):
                nc.vector.tensor_scalar_add(
                    KT[b][:, c * 512:(c + 1) * 512],
                    kproj_chunk_tail.kp[:], bk_sb[:]
                )
                for sub in range(4):
                    tcx = c * 4 + sub
                    nc.sync.dma_start_transpose(
                        KN[b][:, tcx, :], KT[b][:, tcx * 128:(tcx + 1) * 128]
                    )

            def kproj_chunk(b, xts, c):
                kp = pkp.tile([128, 512], F32, tag="pk")
                for dc in range(8):
                    nc.tensor.matmul(
                        kp[:],
                        lhsT=wk_sb[:, dc, :],
                        rhs=xts[c][:, dc, :],
                        start=(dc == 0),
                        stop=(dc == 7),
                    )
                kproj_chunk_tail.kp = kp
                kproj_chunk_tail(b, c)

            def kproj_filler(b, xts):
                """Generator: one PE matmul per next() so batch-1 kproj
                drips into batch-0 attention kb slots without bubbles."""
                for c in range(4):
                    kp = pkp.tile([128, 512], F32, tag="pk")
                    for dc in range(8):
                        nc.tensor.matmul(
                            kp[:],
                            lhsT=wk_sb[:, dc, :],
                            rhs=xts[c][:, dc, :],
                            start=(dc == 0),
                            stop=(dc == 7),
                        )
                        if dc == 7:
                            kproj_chunk_tail.kp = kp
                            kproj_chunk_tail(b, c)
                        yield

            def attn_segment(b, qs, h, ot, filler):
                q0 = qs * QS
                hp = 64 * h

                def S(kb):
                    sq = spool.tile([128, QS], F32, tag="sq")
                    for qc in range(2):
                        nc.tensor.matmul(
                            sq[:, qc * 512:(qc + 1) * 512],
                            lhsT=KT[b][hp:hp + 64, kb * 128:(kb + 1) * 128],
                            rhs=KT[b][hp:hp + 64, q0 + qc * 512:q0 + (qc + 1) * 512],
                            start=True,
                            stop=True,
                            tile_position=(hp, 0),
                        )
                    return sq

                sq_prev = S(0)
                for kb in range(KB):
                    ft = fpool.tile([128, QS], BF16, tag="f")
                    nc.scalar.activation(
                        ft[:], sq_prev[:],
                        mybir.ActivationFunctionType.Exp,
                        scale=0.125,
                        accum_out=saccs[b][:, h, kb, qs:qs + 1],
                    )
                    if kb < KB - 1:
                        sq_prev = S(kb + 1)
                    for qc in range(2):
                        nc.tensor.matmul(
                            ot[hp:hp + 64, qc * 512:(qc + 1) * 512],
                            lhsT=KN[b][:, kb, hp:hp + 64],
                            rhs=ft[:, qc * 512:(qc + 1) * 512],
                            start=(kb == 0),
                            stop=(kb == KB - 1),
                            tile_position=(0, hp),
                        )
                    if filler is not None:
                        next(filler, None)

            def attention(b, filler=None, skip_first_seg=True):
                seg = 0
                for qs in range(L // QS):
                    ot = otp.tile([128, QS], F32, tag="ot")
                    for h in range(HPC):
                        use = None if (filler is None or (skip_first_seg and seg == 0)) else filler
                        attn_segment(b, qs, h, ot, use)
                        seg += 1
                    nc.vector.tensor_copy(OTn[b][:, qs * QS:(qs + 1) * QS], ot[:])

            def normalize(b):
                ss = small.tile([128, HPC * KB], F32, tag="ss")
                nc.vector.tensor_add(
                    ss[:].rearrange("p (h k) -> p h k", h=HPC),
                    saccs[b][:, :, :, 0], saccs[b][:, :, :, 1]
                )
                rc = small.tile([128, HPC * KB], F32, tag="rc")
                nc.vector.reciprocal_approx_fast(rc[:], ss[:])
                rcb = small.tile([128, HPC * KB], BF16, tag="rcb")
                nc.vector.tensor_copy(rcb[:], rc[:])
                tp = pkp.tile([128, 512], BF16, tag="pk")
                nc.tensor.transpose(tp[:HPC * KB, :128], rcb[:], identb[:])
                rsT = small.tile([HPC * KB, 128], BF16, tag="rst")
                nc.vector.tensor_copy(rsT[:], tp[:HPC * KB, :128])
                nc.sync.dma_start(rsd[b][:], rsT[:])
                rr = rrp.tile([128, L], BF16, tag="rr")
                for h in range(HPC):
                    nc.sync.dma_start(
                        rr[64 * h:64 * (h + 1), :],
                        rsd[b][KB * h:KB * (h + 1), :]
                        .rearrange("a c -> (a c)").partition_broadcast(64),
                    )
                for h in range(HPC):
                    nc.vector.tensor_mul(
                        OTn[b][64 * h:64 * (h + 1), :],
                        OTn[b][64 * h:64 * (h + 1), :],
                        rr[64 * h:64 * (h + 1), :],
                    )

            def comm(b, hf):
                nc.sync.dma_start(
                    cc_in[b][hf].rearrange("(j p) t -> p j t", p=128),
                    OTn[b].rearrange("p (j u t) -> p j u t", j=N_CORES, u=NH)[:, :, hf, :],
                )
                nc.gpsimd.collective_compute(
                    "AllToAll",
                    mybir.AluOpType.bypass,
                    replica_groups=[list(range(N_CORES))],
                    ins=[cc_in[b][hf].opt()],
                    outs=[cc_out[b][hf].opt()],
                )

            def proj(b, hf):
                pt = ptp.tile([128, 8, TH], BF16, tag="pt")
                nc.sync.dma_start(
                    pt[:], cc_out[b][hf].rearrange("(po p) t -> p po t", p=128)
                )
                y_t = yp.tile([128, D], F32, tag="y")
                for nc2 in range(D // 512):
                    pj = pkp.tile([128, 512], F32, tag="pk")
                    for dc in range(8):
                        nc.tensor.matmul(
                            pj[:],
                            lhsT=pt[:, dc, :],
                            rhs=wp_sb[:, dc, nc2 * 512:(nc2 + 1) * 512],
                            start=(dc == 0),
                            stop=(dc == 7),
                        )
                    nc.vector.tensor_add(
                        y_t[:, nc2 * 512:(nc2 + 1) * 512],
                        pj[:],
                        bp_rep[:, nc2 * 512:(nc2 + 1) * 512],
                    )
                nc.sync.dma_start(out.ap()[b, hf * TH:(hf + 1) * TH, :], y_t[:])

            # ---- schedule (emission order = per-engine stream order) ----
            xts0 = load_xt(0)
            for c in range(4):
                kproj_chunk(0, xts0, c)
            xts1 = load_xt(1)
            attention(0, filler=kproj_filler(1, xts1))
            normalize(0)
            comm(0, 0)
            comm(0, 1)
            attention(1)
            normalize(1)
            comm(1, 0)
            comm(1, 1)
            proj(0, 0)
            proj(0, 1)
            proj(1, 0)
            proj(1, 1)

    nc.compile()
    return nc


_CACHED = None


def _get_nc():
    global _CACHED
    if _CACHED is None:
        _CACHED = build()
    return _CACHED


def run(inputs, trace=False):
    import ml_dtypes

    bf16 = ml_dtypes.bfloat16
    x = np.asarray(inputs["x"], np.float32)
    W_attn = np.asarray(inputs["W_attn"], np.float32)
    b_attn = np.asarray(inputs["b_attn"], np.float32)
    W_proj = np.asarray(inputs["W_proj"], np.float32)
    b_proj = np.asarray(inputs["b_proj"], np.float32)

    xt = np.ascontiguousarray(x.transpose(0, 2, 1)).astype(bf16)  # [B, D, L]
    wp16 = W_proj.astype(bf16)
    in_maps = []
    for i in range(N_CORES):
        c0 = D + i * DC
        in_maps.append(
            {
                "xt": xt,
                "wk": np.ascontiguousarray(W_attn[:, c0:c0 + DC]).astype(bf16),
                "bk": np.ascontiguousarray(b_attn[c0:c0 + DC].reshape(DC, 1)),
                "wp": wp16,
                "bp": b_proj,
            }
        )

    nc = _get_nc()
    res = run_bass_kernel_spmd(
        nc, in_maps, core_ids=list(range(N_CORES)), trace=trace
    )
    outs = np.stack([res.results[i]["out"] for i in range(N_CORES)])  # [8, B, TPB, D]
    y = outs.transpose(1, 0, 2, 3).reshape(B, L, D)
    return y, res


def kernel(**inputs) -> np.ndarray:
    y, _ = run(inputs)
    return y


# revision 14
# speedup vs baseline: 1.0237x; 1.0237x over previous
"""Distributed Bass kernel for nn_Attention (dense transformer block with the
q=k=v source bug) on 8 TRN2 NeuronCores.

Sharding: tensor-parallel over heads (Megatron-style). Core i owns heads
(2i, 2i+1). Per core:
  1. kproj: KT[d, t] = (x @ W_k_slice).T from host-transposed 1MB x^T chunks;
     KN[k-token, d] via DMA-xbar transposes (off the PE).
  2. Attention per (batch, 1024-query-span, head, 128-key-block): S-pair
     matmuls (2x N=512, f32 PSUM bank limit), one N=1024 Exp activation
     whose accum_out gives softmax denominators for free (E is symmetric
     since q=k, so free-axis row sums == column sums), then O^T
     accumulation col-packed 2 heads per PSUM tile. The kb loop is
     software-pipelined in EMISSION order (S(kb+1) before O(kb)) because
     engine streams execute strictly in order - otherwise the PE stalls at
     O(kb)'s wait-for-exp while S(kb+1) is ready behind it. ACT is the
     roofline: 128 exps x 1.11us + 128 accum drains x 0.28us ~= 178us.
  3. kproj of batch 1 is drip-fed one matmul per kb slot inside batch 0's
     attention segments so it never bubbles the exp stream.
  4. Normalize per batch: fast reciprocal, PE transpose, DRAM
     partition-broadcast, in-place multiply.
  5. AllToAll per (batch, 128-token half); out-projections are emitted
     after the comms so they execute during the A2A waits (HAM warm).

PSUM: S 2x2 banks, ot 2, kproj/proj scratch 2x1.
"""

import numpy as np

import concourse.bass as bass
import concourse.tile as tile
from concourse import bacc, mybir
from concourse.bass_utils import run_bass_kernel_spmd
from concourse.masks import make_identity

N_CORES = 8
B, L, D = 2, 2048, 1024
H, HD = 16, 64
HPC = H // N_CORES  # heads per core = 2
DC = HPC * HD  # head-dim columns per core = 128
TPB = L // N_CORES  # tokens per batch per core (proj stage) = 256
F32 = mybir.dt.float32
BF16 = mybir.dt.bfloat16
QS = 1024  # query span per (head, kb) exp tile
KB = L // 128  # 16 key blocks per batch
NH = 2  # token halves per batch for the A2A/proj pipeline
TH = TPB // NH  # 128 tokens per half


def build():
    nc = bacc.Bacc("TRN2", target_bir_lowering=False, debug=False, num_devices=N_CORES)
    xt = nc.dram_tensor("xt", [B, D, L], BF16, kind="ExternalInput")
    wk = nc.dram_tensor("wk", [D, DC], BF16, kind="ExternalInput")
    bk = nc.dram_tensor("bk", [DC, 1], F32, kind="ExternalInput")
    wp = nc.dram_tensor("wp", [D, D], BF16, kind="ExternalInput")
    bp = nc.dram_tensor("bp", [D], F32, kind="ExternalInput")
    out = nc.dram_tensor("out", [B, TPB, D], F32, kind="ExternalOutput")

    xt_v = xt.ap().rearrange("b (dc p) t -> b p dc t", p=128)  # [B, 128, 8, L]
    wk_v = wk.ap().rearrange("(dc p) m -> p dc m", p=128)  # [128, 8, DC]
    wp_v = wp.ap().rearrange("(dc p) n -> p dc n", p=128)  # [128, 8, D]

    with tile.TileContext(nc) as tc:
        with (
            tc.tile_pool(name="consts", bufs=1) as consts,
            tc.tile_pool(name="big", bufs=1) as big,
            tc.tile_pool(name="xtp", bufs=8) as xtp,
            tc.tile_pool(name="fpool", bufs=3) as fpool,
            tc.tile_pool(name="small", bufs=4) as small,
            tc.tile_pool(name="rrp", bufs=2) as rrp,
            tc.tile_pool(name="ptp", bufs=2) as ptp,
            tc.tile_pool(name="yp", bufs=2) as yp,
            tc.tile_pool(name="spool", bufs=2, space="PSUM") as spool,
            tc.tile_pool(name="otp", bufs=1, space="PSUM") as otp,
            tc.tile_pool(name="pkp", bufs=2, space="PSUM") as pkp,
            tc.tile_pool(name="dram", bufs=1, space="DRAM") as dram,
        ):
            # ---- constants ----
            wk_sb = consts.tile([128, 8, DC], BF16)
            nc.sync.dma_start(wk_sb[:], wk_v)
            bk_sb = consts.tile([128, 1], F32)
            nc.sync.dma_start(bk_sb[:], bk.ap())
            wp_sb = consts.tile([128, 8, D], BF16)
            nc.sync.dma_start(wp_sb[:], wp_v)
            bp_rep = consts.tile([128, D], F32)
            nc.sync.dma_start(bp_rep[:], bp.ap().partition_broadcast(128))
            identb = consts.tile([128, 128], BF16)
            make_identity(nc, identb[:])

            # persistent activations
            KT = [big.tile([128, L], BF16, name=f"kt{b}") for b in range(B)]
            KN = [big.tile([128, KB, 128], BF16, name=f"kn{b}") for b in range(B)]
            OTn = [big.tile([128, L], BF16, name=f"otn{b}") for b in range(B)]
            saccs = [big.tile([128, HPC, KB, 2], F32, name=f"sacc{b}") for b in range(B)]

            rsd = [dram.tile([HPC * KB, 128], BF16, name=f"rsd{b}") for b in range(B)]
            cc_in = [dram.tile([N_CORES * 128, TPB], BF16, name=f"cc_in{b}")
                     for b in range(B)]
            cc_out = [dram.tile([N_CORES * 128, TPB], BF16, name=f"cc_out{b}")
                      for b in range(B)]

            def load_xt(b):
                tiles = []
                for c in range(4):
                    xt_t = xtp.tile([128, 8, 512], BF16, tag="xt")
                    nc.gpsimd.dma_start(xt_t[:], xt_v[b, :, :, c * 512:(c + 1) * 512])
                    tiles.append(xt_t)
                return tiles

            state = {}

            def kproj_tail(b, c, kp):
                nc.vector.tensor_scalar_add(
                    KT[b][:, c * 512:(c + 1) * 512], kp[:], bk_sb[:]
                )
                for sub in range(4):
                    tcx = c * 4 + sub
                    nc.sync.dma_start_transpose(
                        KN[b][:, tcx, :], KT[b][:, tcx * 128:(tcx + 1) * 128]
                    )

            def kproj_chunk(b, xts, c):
                kp = pkp.tile([128, 512], F32, tag="pk")
                for dc in range(8):
                    nc.tensor.matmul(
                        kp[:],
                        lhsT=wk_sb[:, dc, :],
                        rhs=xts[c][:, dc, :],
                        start=(dc == 0),
                        stop=(dc == 7),
                    )
                kproj_tail(b, c, kp)

            def kproj_microops(fill, b, xts, chunks):
                # each micro-op emits ONE kproj matmul; chunk tail rides dc==7
                for c in chunks:
                    for dc in range(8):
                        def op(b=b, xts=xts, c=c, dc=dc):
                            if dc == 0:
                                state[("kp", b, c)] = pkp.tile(
                                    [128, 512], F32, tag="pk", name=f"kpf{b}_{c}")
                            kp = state[("kp", b, c)]
                            nc.tensor.matmul(
                                kp[:],
                                lhsT=wk_sb[:, dc, :],
                                rhs=xts[c][:, dc, :],
                                start=(dc == 0),
                                stop=(dc == 7),
                            )
                            if dc == 7:
                                kproj_tail(b, c, kp)
                        fill.append(op)

            def attn_segment(b, qs, h, ot, filler, pops):
                q0 = qs * QS
                hp = 64 * h

                def S(kb):
                    sq = spool.tile([128, QS], F32, tag="sq")
                    for qc in range(2):
                        nc.tensor.matmul(
                            sq[:, qc * 512:(qc + 1) * 512],
                            lhsT=KT[b][hp:hp + 64, kb * 128:(kb + 1) * 128],
                            rhs=KT[b][hp:hp + 64, q0 + qc * 512:q0 + (qc + 1) * 512],
                            start=True,
                            stop=True,
                            tile_position=(hp, 0),
                        )
                    return sq

                sq_prev = S(0)
                for kb in range(KB):
                    ft = fpool.tile([128, QS], BF16, tag="f")
                    nc.scalar.activation(
                        ft[:], sq_prev[:],
                        mybir.ActivationFunctionType.Exp,
                        scale=0.125,
                        accum_out=saccs[b][:, h, kb, qs:qs + 1],
                    )
                    if kb < KB - 1:
                        sq_prev = S(kb + 1)
                    for qc in range(2):
                        nc.tensor.matmul(
                            ot[hp:hp + 64, qc * 512:(qc + 1) * 512],
                            lhsT=KN[b][:, kb, hp:hp + 64],
                            rhs=ft[:, qc * 512:(qc + 1) * 512],
                            start=(kb == 0),
                            stop=(kb == KB - 1),
                            tile_position=(0, hp),
                        )
                    for _ in range(pops):
                        if filler:
                            filler.popleft()()

            def attention(b, filler=None, pops_seg0=0, pops=0):
                seg = 0
                for qs in range(L // QS):
                    ot = otp.tile([128, QS], F32, tag="ot")
                    for h in range(HPC):
                        p = pops_seg0 if seg == 0 else pops
                        attn_segment(b, qs, h, ot, filler, p)
                        seg += 1
                    nc.vector.tensor_copy(OTn[b][:, qs * QS:(qs + 1) * QS], ot[:])

            def normalize(b):
                ss = small.tile([128, HPC * KB], F32, tag="ss")
                nc.vector.tensor_add(
                    ss[:].rearrange("p (h k) -> p h k", h=HPC),
                    saccs[b][:, :, :, 0], saccs[b][:, :, :, 1]
                )
                rc = small.tile([128, HPC * KB], F32, tag="rc")
                nc.vector.reciprocal_approx_fast(rc[:], ss[:])
                rcb = small.tile([128, HPC * KB], BF16, tag="rcb")
                nc.vector.tensor_copy(rcb[:], rc[:])
                tp = pkp.tile([128, 512], BF16, tag="pk")
                nc.tensor.transpose(tp[:HPC * KB, :128], rcb[:], identb[:])
                rsT = small.tile([HPC * KB, 128], BF16, tag="rst")
                nc.vector.tensor_copy(rsT[:], tp[:HPC * KB, :128])
                nc.sync.dma_start(rsd[b][:], rsT[:])
                rr = rrp.tile([128, L], BF16, tag="rr")
                for h in range(HPC):
                    nc.sync.dma_start(
                        rr[64 * h:64 * (h + 1), :],
                        rsd[b][KB * h:KB * (h + 1), :]
                        .rearrange("a c -> (a c)").partition_broadcast(64),
                    )
                for h in range(HPC):
                    nc.vector.tensor_mul(
                        OTn[b][64 * h:64 * (h + 1), :],
                        OTn[b][64 * h:64 * (h + 1), :],
                        rr[64 * h:64 * (h + 1), :],
                    )

            def comm(b):
                nc.sync.dma_start(
                    cc_in[b].rearrange("(j p) t -> p j t", p=128),
                    OTn[b].rearrange("p (j t) -> p j t", j=N_CORES),
                )
                nc.gpsimd.collective_compute(
                    "AllToAll",
                    mybir.AluOpType.bypass,
                    replica_groups=[list(range(N_CORES))],
                    ins=[cc_in[b].opt()],
                    outs=[cc_out[b].opt()],
                )

            def proj(b, hf):
                pt = ptp.tile([128, 8, TH], BF16, tag="pt")
                nc.sync.dma_start(
                    pt[:],
                    cc_out[b].rearrange(
                        "(po p) (u t) -> p po u t", p=128, u=NH)[:, :, hf, :],
                )
                y_t = yp.tile([128, D], F32, tag="y")
                for nc2 in range(D // 512):
                    pj = pkp.tile([128, 512], F32, tag="pk")
                    for dc in range(8):
                        nc.tensor.matmul(
                            pj[:],
                            lhsT=pt[:, dc, :],
                            rhs=wp_sb[:, dc, nc2 * 512:(nc2 + 1) * 512],
                            start=(dc == 0),
                            stop=(dc == 7),
                        )
                    nc.vector.tensor_add(
                        y_t[:, nc2 * 512:(nc2 + 1) * 512],
                        pj[:],
                        bp_rep[:, nc2 * 512:(nc2 + 1) * 512],
                    )
                nc.sync.dma_start(out.ap()[b, hf * TH:(hf + 1) * TH, :], y_t[:])

            # ---- schedule (emission order = per-engine stream order) ----
            from collections import deque
            xts0 = load_xt(0)
            xts1 = load_xt(1)
            kproj_chunk(0, xts0, 0)
            kproj_chunk(0, xts0, 1)
            fill = deque()
            kproj_microops(fill, 0, xts0, [2, 3])
            kproj_microops(fill, 1, xts1, [0, 1, 2, 3])
            attention(0, filler=fill, pops_seg0=2, pops=1)
            normalize(0)
            comm(0)
            attention(1)
            normalize(1)
            proj(0, 0)
            comm(1)
            proj(0, 1)
            proj(1, 0)
            proj(1, 1)

    nc.compile()
    return nc


_CACHED = None


def _get_nc():
    global _CACHED
    if _CACHED is None:
        _CACHED = build()
    return _CACHED


def run(inputs, trace=False):
    import ml_dtypes

    bf16 = ml_dtypes.bfloat16
    x = np.asarray(inputs["x"], np.float32)
    W_attn = np.asarray(inputs["W_attn"], np.float32)
    b_attn = np.asarray(inputs["b_attn"], np.float32)
    W_proj = np.asarray(inputs["W_proj"], np.float32)
    b_proj = np.asarray(inputs["b_proj"], np.float32)

    xt = np.ascontiguousarray(x.transpose(0, 2, 1)).astype(bf16)  # [B, D, L]
    wp16 = W_proj.astype(bf16)
    in_maps = []
    for i in range(N_CORES):
        c0 = D + i * DC
        in_maps.append(
            {
                "xt": xt,
                "wk": np.ascontiguousarray(W_attn[:, c0:c0 + DC]).astype(bf16),
                "bk": np.ascontiguousarray(b_attn[c0:c0 + DC].reshape(DC, 1)),
                "wp": wp16,
                "bp": b_proj,
            }
        )

    nc = _get_nc()
    res = run_bass_kernel_spmd(
        nc, in_maps, core_ids=list(range(N_CORES)), trace=trace
    )
    outs = np.stack([res.results[i]["out"] for i in range(N_CORES)])  # [8, B, TPB, D]
    y = outs.transpose(1, 0, 2, 3).reshape(B, L, D)
    return y, res


def kernel(**inputs) -> np.ndarray:
    y, _ = run(inputs)
    return y
